# revision 1
# baseline (speedup 1.0000x reference)
"""Trainium2 Bass kernel for nn_CLConv (gnn_message_passing).

Contract: kernel(**inputs) takes FULL unsharded inputs, returns the FULL
output (4096, 32*max_view) float32.  Internally shards row-wise across the
8 NeuronCores (each core owns N/8=512 destination rows and the matching
512x4096 slice of `geodesic`).

Algorithmic shape: the dense (N,N) kernel matrix built by the scatter has at
most K=16 non-zeros per row (the row's edges).  So (kernel**p) @ x collapses
to per-edge weights w_e = lck_e * exp(-alpha_e * geo[r,c]) * angle, with
duplicate (row,col) edges combined exactly like scatter-add, and
out[r] = sum_k w^p[r,k] * x[col[r,k]].  Only 16 geodesic values per row are
needed -> indirect-DMA gather instead of streaming the dense 64MB matrix.

Device work per core: edge MLP in feature-on-partition layout -- layers 0
and 1 compose linearly (no activation between them in the reference), so the
host folds W01 = W0@W1, b01 = b0@W1 + b1 and the device runs a single K=5
matmul layer (bias via a ones row) + tanh + BN*tanh + the W2 layer.
Attention is reduced to a per-edge bilinear form, duplicates combined via
masked shifted adds, geodesic/x fetched with dma_gather, and a weighted
16->1 tree reduction produces each destination row.
"""

import sys

sys.path.insert(0, "/opt/trn_rl_repo")

import numpy as np
np_bf16 = np.float16

import concourse.bacc as bacc
import concourse.bass as bass
import concourse.mybir as mybir
import concourse.tile as tile

# problem constants (hardcoded per harness contract)
N = 4096
KN = 16           # neighbours per node
F = 32            # node feature dim
H = 64            # MLP hidden
NCORES = 8
NLOC = N // NCORES          # 512 rows per core
ELOC = NLOC * KN            # 8192 edges per core
P = 128                     # SBUF partitions
JW = ELOC // P              # 64 edge slots per partition (EP layout)
CHUNKS = 16                 # MLP chunks
CW = ELOC // CHUNKS         # 512 edges per chunk
BN_EPS = 1e-5

f32 = mybir.dt.float32
bf16 = mybir.dt.float16
i32 = mybir.dt.int32


def _build_program(dmax, n_views, attn_consts, sim=False):
    """One SPMD Bass program (identical across cores; data differs)."""
    nc = bacc.Bacc("TRN2", target_bir_lowering=False, debug=False,
                   num_swdge_queues=2)

    geo = nc.dram_tensor("geo", [NLOC * N // 64, 64], f32, kind="ExternalInput")
    xtab = nc.dram_tensor("xtab", [N, 64], f32, kind="ExternalInput")
    gidx = nc.dram_tensor("gidx", [P, 2 * ELOC // 16], mybir.dt.int16, kind="ExternalInput")
    coor_ep = nc.dram_tensor("coor_ep", [P, 4 * JW], f32, kind="ExternalInput")
    qcoor = nc.dram_tensor("qcoor", [P, 16], f32, kind="ExternalInput")
    colx = nc.dram_tensor("colx", [P, JW], i32, kind="ExternalInput")
    repm = nc.dram_tensor("repm", [P, JW], f32, kind="ExternalInput")
    dupm = None
    if dmax > 0:
        dupm = nc.dram_tensor("dupm", [P, 2 * dmax * JW], f32, kind="ExternalInput")
    w01b = nc.dram_tensor("w01b", [5, H], bf16, kind="ExternalInput")  # [W0@W1; b0@W1+b1]
    w2oh = nc.dram_tensor("w2oh", [P, CHUNKS * CHUNKS], bf16, kind="ExternalInput")
    bn_sc = nc.dram_tensor("bn_sc", [P, 2], f32, kind="ExternalInput")  # scale,bias
    b2rep = nc.dram_tensor("b2rep", [CHUNKS, 1], f32, kind="ExternalInput")
    onesr = nc.dram_tensor("onesr", [1, ELOC], bf16, kind="ExternalInput")
    vals_bounce = nc.dram_tensor("vals_bounce", [ELOC, 1], f32)
    th_bounce = nc.dram_tensor("th_bounce", [P, 4 * JW], bf16)
    out = nc.dram_tensor("out", [NLOC, n_views * F], f32, kind="ExternalOutput")

    AB = mybir.AluOpType
    AF = mybir.ActivationFunctionType
    a00, a01, a10, a11, u0, u1, v0, v1, w0s = [float(z) for z in attn_consts]

    with tile.TileContext(nc) as tc:
        with (
            tc.tile_pool(name="sbuf", bufs=1) as sb,
            tc.tile_pool(name="sbuf2", bufs=3) as sb2,
            tc.tile_pool(name="psum", bufs=6, space="PSUM") as ps,
            tc.tile_pool(name="psumv", bufs=1, space="PSUM") as psv,
        ):
            # ---------------- gathers (independent of MLP) ----------------
            # dma_gather layout: logical index i -> out[i%128, i//128, :];
            # host permutes index lists so out[p, j, :] is EP edge (p, j).
            SW = ELOC // 16
            gidx_t = sb.tile([P, 2 * SW], mybir.dt.int16)
            nc.sync.dma_start(out=gidx_t[:], in_=gidx[:])
            colx_t = sb.tile([P, JW], i32)
            nc.sync.dma_start(out=colx_t[:], in_=colx[:])

            gb_t = sb.tile([P, JW * 64], f32)
            nc.gpsimd.dma_gather(
                gb_t[:].rearrange("p (j c) -> p j c", c=64), geo[:],
                gidx_t[:, SW:2 * SW], ELOC, ELOC, 64, single_packet=False)

            xg_t = sb.tile([P, JW * 64], f32)
            nc.gpsimd.dma_gather(
                xg_t[:].rearrange("p (j c) -> p j c", c=64), xtab[:],
                gidx_t[:, 0:SW], ELOC, ELOC, 64, single_packet=False, queue_num=1)

            # extract geo value: one-hot over the 64-block at col%64
            cmod_t = sb.tile([P, JW], i32)
            nc.vector.tensor_scalar(out=cmod_t[:], in0=colx_t[:], scalar1=63,
                                    scalar2=None, op0=AB.bitwise_and)
            iot_t = sb.tile([P, 64], i32)
            nc.gpsimd.iota(iot_t[:], pattern=[[1, 64]], base=0, channel_multiplier=0)
            oh_t = sb.tile([P, JW * 64], bf16)
            ohv = oh_t[:].rearrange("p (j c) -> p j c", c=64)
            nc.vector.tensor_tensor(
                out=ohv, in0=cmod_t[:, :, None].to_broadcast([P, JW, 64]),
                in1=iot_t[:, None, :].to_broadcast([P, JW, 64]), op=AB.is_equal)
            nc.vector.tensor_tensor(out=ohv, in0=gb_t[:].rearrange("p (j c) -> p j c", c=64),
                                    in1=ohv, op=AB.mult)
            geo_t = sb.tile([P, JW], f32)
            nc.vector.tensor_reduce(out=geo_t[:], in_=ohv, axis=mybir.AxisListType.X,
                                    op=AB.add)

            # ---------------- MLP: vals ----------------
            # tanh on the edge-partition coor tile, then repartition to (4, ELOC)
            ce_t = sb.tile([P, 4 * JW], f32)
            nc.sync.dma_start(out=ce_t[:], in_=coor_ep[:])
            thep_t = sb.tile([P, 4 * JW], bf16)
            nc.scalar.activation(out=thep_t[:], in_=ce_t[:], func=AF.Tanh)
            nc.sync.dma_start(out=th_bounce[:], in_=thep_t[:])
            th2_t = sb.tile([5, ELOC], bf16)
            tbv = th_bounce[:].rearrange("p (j c) -> c p j", c=4)
            t2v = th2_t[0:4, :].rearrange("c (p j) -> c p j", p=P)
            for c_, eng in enumerate((nc.sync, nc.scalar, nc.sync, nc.scalar)):
                eng.dma_start(out=t2v[c_:c_ + 1], in_=tbv[c_:c_ + 1])
            nc.sync.dma_start(out=th2_t[4:5, :], in_=onesr[:])

            w01b_t = sb.tile([5, H], bf16)
            nc.sync.dma_start(out=w01b_t[:], in_=w01b[:])
            w2oh_t = sb.tile([P, CHUNKS * CHUNKS], bf16)
            nc.sync.dma_start(out=w2oh_t[:], in_=w2oh[:])
            bnsc_t = sb.tile([P, 2], f32)
            nc.sync.dma_start(out=bnsc_t[:], in_=bn_sc[:])
            b2_t = sb.tile([CHUNKS, 1], f32)
            nc.sync.dma_start(out=b2_t[:], in_=b2rep[:])

            h2_t = sb.tile([P, ELOC // 2], bf16)
            for i in range(CHUNKS):
                pt = ps.tile([H, CW], f32, name="pt", tag="ps")
                nc.tensor.matmul(out=pt[:], lhsT=w01b_t[:],
                                 rhs=th2_t[:, CW * i:CW * (i + 1)], start=True, stop=True)
                po, fo = (i // 8) * H, (i % 8) * CW
                nc.scalar.activation(out=h2_t[po:po + H, fo:fo + CW], in_=pt[:],
                                     func=AF.Tanh)

            h4_t = sb.tile([P, ELOC // 2], bf16)
            for i in range(8):
                fo = i * CW
                nc.scalar.activation(out=h4_t[:, fo:fo + CW], in_=h2_t[:, fo:fo + CW],
                                     func=AF.Tanh, scale=bnsc_t[:, 0:1],
                                     bias=bnsc_t[:, 1:2])

            # two accumulation groups: matmul groups must not switch base
            # partition mid-group (HW fault) -> one group per partition half
            vpa_t = psv.tile([CHUNKS, CW], f32)
            vpb_t = psv.tile([CHUNKS, CW], f32)
            for i in range(CHUNKS):
                po, fo = (i // 8) * H, (i % 8) * CW
                vp = vpa_t if i < 8 else vpb_t
                nc.tensor.matmul(out=vp[:],
                                 lhsT=w2oh_t[po:po + H, CHUNKS * i:CHUNKS * (i + 1)],
                                 rhs=h4_t[po:po + H, fo:fo + CW],
                                 start=(i % 8 == 0), stop=(i % 8 == 7))
            vals16_t = sb.tile([CHUNKS, CW], f32)
            nc.vector.tensor_scalar(out=vals16_t[:], in0=vpa_t[:],
                                    scalar1=b2_t[:, 0:1], scalar2=None, op0=AB.add)
            nc.vector.tensor_tensor(out=vals16_t[:], in0=vals16_t[:], in1=vpb_t[:],
                                    op=AB.add)
            nc.vector.tensor_scalar(out=vals16_t[:], in0=vals16_t[:], scalar1=0.0,
                                    scalar2=None, op0=AB.max)

            # bridge (16,512) -> (128,64) via DRAM bounce
            nc.scalar.dma_start(
                out=vals_bounce[:].rearrange("(c q) o -> c (q o)", c=CHUNKS),
                in_=vals16_t[:])
            vals_t = sb.tile([P, JW], f32)
            nc.scalar.dma_start(out=vals_t[:],
                                in_=vals_bounce[:].rearrange("(p j) o -> p (j o)", p=P))

            # ---------------- attention alpha (EP layout) ----------------
            qc_t = sb.tile([P, 16], f32)
            nc.sync.dma_start(out=qc_t[:], in_=qcoor[:])

            def comp(t, n, c):  # (P, n*4) tile -> (P, n, 1) view of component c
                return t[:].rearrange("p (j c) -> p j c", c=4)[:, :, c:c + 1]

            def flat3(t, n):    # (P, n) tile -> (P, n, 1) view
                return t[:].rearrange("p (j o) -> p j o", o=1)

            sx_t = sb.tile([P, JW], f32)
            nc.vector.tensor_tensor(out=flat3(sx_t, JW), in0=comp(ce_t, JW, 0),
                                    in1=comp(ce_t, JW, 2), op=AB.add)
            sy_t = sb.tile([P, JW], f32)
            nc.vector.tensor_tensor(out=flat3(sy_t, JW), in0=comp(ce_t, JW, 1),
                                    in1=comp(ce_t, JW, 3), op=AB.add)
            sqx_t = sb.tile([P, 4], f32)
            nc.vector.tensor_tensor(out=flat3(sqx_t, 4), in0=comp(qc_t, 4, 0),
                                    in1=comp(qc_t, 4, 2), op=AB.add)
            sqy_t = sb.tile([P, 4], f32)
            nc.vector.tensor_tensor(out=flat3(sqy_t, 4), in0=comp(qc_t, 4, 1),
                                    in1=comp(qc_t, 4, 3), op=AB.add)

            def qcoef(out_t, cx, cy, cc):
                tq_t = sb.tile([P, 4], f32, tag="tq")
                nc.vector.tensor_scalar(out=out_t[:], in0=sqx_t[:], scalar1=cx,
                                        scalar2=cc, op0=AB.mult, op1=AB.add)
                nc.vector.tensor_scalar(out=tq_t[:], in0=sqy_t[:], scalar1=cy,
                                        scalar2=None, op0=AB.mult)
                nc.vector.tensor_tensor(out=out_t[:], in0=out_t[:], in1=tq_t[:], op=AB.add)

            qa_t = sb.tile([P, 4], f32)
            qcoef(qa_t, a00, a10, v0)
            qb_t = sb.tile([P, 4], f32)
            qcoef(qb_t, a01, a11, v1)
            qc2_t = sb.tile([P, 4], f32)
            qcoef(qc2_t, u0, u1, w0s)

            def bview(t):   # (P,4) -> stride-0 broadcast (P,4,16)
                return t[:, :, None].to_broadcast([P, 4, KN])

            def gview(t):   # (P,64) -> (P,4,16)
                return t[:].rearrange("p (g k) -> p g k", k=KN)

            alpha_t = sb.tile([P, JW], f32)
            tmp_t = sb.tile([P, JW], f32)
            nc.vector.tensor_tensor(out=gview(alpha_t), in0=gview(sx_t),
                                    in1=bview(qa_t), op=AB.mult)
            nc.vector.tensor_tensor(out=gview(tmp_t), in0=gview(sy_t),
                                    in1=bview(qb_t), op=AB.mult)
            nc.vector.tensor_tensor(out=alpha_t[:], in0=alpha_t[:], in1=tmp_t[:], op=AB.add)
            nc.vector.tensor_tensor(out=gview(alpha_t), in0=gview(alpha_t),
                                    in1=bview(qc2_t), op=AB.add)
            nc.vector.tensor_scalar(out=tmp_t[:], in0=alpha_t[:], scalar1=-1.0,
                                    scalar2=None, op0=AB.mult)
            nc.vector.tensor_tensor(out=alpha_t[:], in0=alpha_t[:], in1=tmp_t[:], op=AB.max)

            # ---------------- duplicate combining ----------------
            rep_t = sb.tile([P, JW], f32)
            nc.sync.dma_start(out=rep_t[:], in_=repm[:])
            vsum_t = sb.tile([P, JW], f32)
            asum_t = sb.tile([P, JW], f32)
            nc.vector.tensor_copy(out=vsum_t[:], in_=vals_t[:])
            nc.vector.tensor_copy(out=asum_t[:], in_=alpha_t[:])
            if dmax > 0:
                dup_t = sb.tile([P, 2 * dmax * JW], f32)
                nc.sync.dma_start(out=dup_t[:], in_=dupm[:])
                st_t = sb.tile([P, JW], f32)
                for d in range(1, dmax + 1):
                    fwd = dup_t[:, (d - 1) * JW:(d - 1) * JW + JW - d]
                    bwd = dup_t[:, (dmax + d - 1) * JW + d:(dmax + d) * JW]
                    for src, dst in ((vals_t, vsum_t), (alpha_t, asum_t)):
                        nc.vector.tensor_tensor(out=st_t[:, 0:JW - d],
                                                in0=src[:, d:JW], in1=fwd, op=AB.mult)
                        nc.vector.tensor_tensor(out=dst[:, 0:JW - d],
                                                in0=dst[:, 0:JW - d],
                                                in1=st_t[:, 0:JW - d], op=AB.add)
                        nc.vector.tensor_tensor(out=st_t[:, d:JW],
                                                in0=src[:, 0:JW - d], in1=bwd, op=AB.mult)
                        nc.vector.tensor_tensor(out=dst[:, d:JW],
                                                in0=dst[:, d:JW],
                                                in1=st_t[:, d:JW], op=AB.add)
            nc.vector.tensor_tensor(out=vsum_t[:], in0=vsum_t[:], in1=rep_t[:], op=AB.mult)

            # ---------------- weights & powers ----------------
            wdec_t = sb.tile([P, JW], f32)
            nc.vector.tensor_tensor(out=wdec_t[:], in0=asum_t[:], in1=geo_t[:], op=AB.mult)
            dec_t = sb.tile([P, JW], f32)
            nc.scalar.activation(out=dec_t[:], in_=wdec_t[:], func=AF.Exp, scale=-1.0)
            w_pow = [sb.tile([P, JW], f32, name="wp0", tag="wp0")]
            nc.vector.tensor_tensor(out=w_pow[0][:], in0=vsum_t[:], in1=dec_t[:], op=AB.mult)
            for v in range(1, n_views):
                wn_t = sb.tile([P, JW], f32, name=f"wp{v}", tag=f"wp{v}")
                nc.vector.tensor_tensor(out=wn_t[:], in0=w_pow[-1][:], in1=w_pow[0][:],
                                        op=AB.mult)
                w_pow.append(wn_t)

            # ---------------- weighted gather-reduce + output ----------------
            xgv = xg_t[:].rearrange("p (g k c) -> p g k c", g=4, k=KN)[:, :, :, 0:F]
            outv = out[:].rearrange("(p g) (v f) -> p g v f", p=P, v=n_views)
            for v in range(n_views):
                pv_t = sb2.tile([P, JW * F], f32, tag="pv")
                pvv = pv_t[:].rearrange("p (g k f) -> p g k f", g=4, k=KN)
                wb = w_pow[v][:].rearrange("p (g k) -> p g k", k=KN)[:, :, :, None] \
                    .to_broadcast([P, 4, KN, F])
                nc.vector.tensor_tensor(out=pvv, in0=xgv, in1=wb, op=AB.mult)
                for hw in (8, 4, 2, 1):
                    nc.vector.tensor_tensor(
                        out=pvv[:, :, 0:hw, :], in0=pvv[:, :, 0:hw, :],
                        in1=pvv[:, :, hw:2 * hw, :], op=AB.add)
                oeng = (nc.sync, nc.scalar, nc.scalar)[v % 3]
                oeng.dma_start(out=outv[:, :, v:v + 1, :], in_=pvv[:, :, 0:1, :])

    nc.compile()
    return nc


def _prepare(inputs):
    """Host-side sharding/staging: index sorting, mask building, weight folds."""
    x = np.asarray(inputs["x"], np.float32)
    coor = np.asarray(inputs["local_graph_coor"], np.float32)
    sparse_idx = np.asarray(inputs["sparse_idx"])
    geodesic = np.asarray(inputs["geodesic"], np.float32)
    angle_ratio = float(np.asarray(inputs["angle_ratio"]).ravel()[0])
    Wq = np.asarray(inputs["Wq"], np.float32); bq = np.asarray(inputs["bq"], np.float32)
    Wk = np.asarray(inputs["Wk"], np.float32); bk = np.asarray(inputs["bk"], np.float32)
    W0 = np.asarray(inputs["W0"], np.float32); b0 = np.asarray(inputs["b0"], np.float32)
    W1 = np.asarray(inputs["W1"], np.float32); b1 = np.asarray(inputs["b1"], np.float32)
    bn_g = np.asarray(inputs["bn_g"], np.float32); bn_b = np.asarray(inputs["bn_b"], np.float32)
    bn_m = np.asarray(inputs["bn_m"], np.float32); bn_v = np.asarray(inputs["bn_v"], np.float32)
    W2 = np.asarray(inputs["W2"], np.float32); b2 = np.asarray(inputs["b2"], np.float32)
    n_views = int(np.asarray(inputs["max_view"]).ravel()[0])

    col = np.asarray(sparse_idx[1], np.int64).reshape(N, KN)
    order = np.argsort(col, axis=1, kind="stable")
    col_s = np.take_along_axis(col, order, axis=1)                  # (N,K)
    eidx = (np.arange(N)[:, None] * KN + order).reshape(-1)
    coor_s = coor[eidx]                                             # (E,4)

    same_prev = np.zeros((N, KN), bool)
    same_prev[:, 1:] = col_s[:, 1:] == col_s[:, :-1]
    rep = (~same_prev).astype(np.float32) * angle_ratio
    run = np.zeros((N, KN), np.int64)
    for k in range(1, KN):
        run[:, k] = np.where(same_prev[:, k], run[:, k - 1] + 1, 0)
    dmax = int(run.max())

    A = Wq @ Wk.T
    u = Wq @ bk
    vv = Wk @ bq
    attn_consts = (A[0, 0], A[0, 1], A[1, 0], A[1, 1], u[0], u[1],
                   vv[0], vv[1], float(bq @ bk))

    bns = (bn_g / np.sqrt(bn_v + BN_EPS)).astype(np.float32)
    bnc = (bn_b - bn_m * bns).astype(np.float32)
    w01b_a = np.vstack([W0 @ W1, (b0 @ W1 + b1)[None, :]]).astype(np.float32)
    bn_sc = np.stack([np.tile(bns, 2), np.tile(bnc, 2)], axis=1)
    b2rep_a = np.full((CHUNKS, 1), float(b2.ravel()[0]), np.float32)
    w2oh_a = np.zeros((H, CHUNKS * CHUNKS), np.float32)
    for i in range(CHUNKS):
        w2oh_a[:, CHUNKS * i + i] = W2[:, 0]
    w2oh_a = np.vstack([w2oh_a, w2oh_a])                            # both halves
    x_pad = np.zeros((N, 64), np.float32)
    x_pad[:, :F] = x

    in_maps = []
    for c in range(NCORES):
        r0 = c * NLOC
        colc = col_s[r0:r0 + NLOC].reshape(-1)
        coorc = coor_s[r0 * KN:(r0 + NLOC) * KN]
        qrows = coor[(np.arange(r0, r0 + NLOC)) * KN]               # orig slot0

        def wrap16(lst):  # dma_gather index layout: i -> [i%16, i//16], x8 replicas
            return np.tile(lst.reshape(ELOC // 16, 16).T, (8, 1)).astype(np.int16)

        colep = colc.reshape(P, JW)
        blkep = (((np.arange(ELOC) // KN) * N + colc) // 64).reshape(P, JW)
        gidx_a = np.hstack([wrap16(colep.T.ravel()), wrap16(blkep.T.ravel())])
        m = {
            "geo": np.ascontiguousarray(geodesic[r0:r0 + NLOC]).reshape(-1, 64),
            "xtab": x_pad,
            "gidx": gidx_a,
            "coor_ep": np.ascontiguousarray(coorc).reshape(P, 4 * JW),
            "qcoor": np.ascontiguousarray(qrows).reshape(P, 16),
            "colx": colc.astype(np.int32).reshape(P, JW),
            "repm": rep[r0:r0 + NLOC].reshape(P, JW).astype(np.float32),
            "w01b": w01b_a.astype(np_bf16), "w2oh": w2oh_a.astype(np_bf16),
            "bn_sc": bn_sc, "b2rep": b2rep_a,
            "onesr": np.ones((1, ELOC), np.float16),
        }
        if dmax > 0:
            dup = np.zeros((P, 2 * dmax * JW), np.float32)
            for d in range(1, dmax + 1):
                fwd = np.zeros(ELOC, np.float32)
                fwd[:ELOC - d] = (((np.arange(ELOC - d) % KN) + d < KN) &
                                  (colc[d:] == colc[:-d]))
                bwd = np.zeros(ELOC, np.float32)
                bwd[d:] = (((np.arange(d, ELOC) % KN) - d >= 0) &
                           (colc[:-d] == colc[d:]))
                dup[:, (d - 1) * JW:d * JW] = fwd.reshape(P, JW)
                dup[:, (dmax + d - 1) * JW:(dmax + d) * JW] = bwd.reshape(P, JW)
            m["dupm"] = dup
        in_maps.append(m)
    return in_maps, dmax, n_views, attn_consts


def kernel(**inputs):
    from concourse.bass_utils import run_bass_kernel_spmd
    in_maps, dmax, n_views, attn_consts = _prepare(inputs)
    nc = _build_program(dmax, n_views, attn_consts, sim=False)
    res = run_bass_kernel_spmd(nc, in_maps, list(range(NCORES)))
    return np.vstack([res.results[c]["out"] for c in range(NCORES)])

